# revision 5
# baseline (speedup 1.0000x reference)
"""Trainium2 Bass kernel for nn_AnalogLayer.

Math (see reference):
    A[p, m] built from cos/sin of (-2*pi/256 * p_values[p%64] * (m%256)),
    y[bn, :] = A @ x[bn, :]  for each of the batch*32 rows of length 512.

Strategy: pure data-parallel over batch across 8 NeuronCores. Per core:
  x_shard [512, 16384] -> viewed as [16384, 512] rows; y_shard [16384, 128].
  - Build A [128, 512] on-device from p_values (iota + range-reduce + Sin LUT)
  - Per group of 512 rows: DMA in (f32), cast to bf16 (ACT), PE-transpose
    x tiles (128x128) into PSUM, DVE-copy to SBUF, 4 accumulating matmuls
    with A^T chunks stationary -> y^T in PSUM, cast to bf16, PE-transpose
    back, copy to f32 SBUF, DMA out.
"""

import math

import numpy as np

import concourse.bacc as bacc
import concourse.bass as bass
import concourse.masks as masks
import concourse.mybir as mybir
import concourse.tile as tile
from concourse.bass_utils import run_bass_kernel_spmd

N_CORES = 8
BATCH = 4096
SHARD = BATCH // N_CORES  # 512 batch rows per core
NBLK = 32                 # blocks per batch row
M2 = 512                  # contraction dim (2*M)
P2 = 128                  # output dim (2*P)
BN = SHARD * NBLK         # 16384 logical rows per core
GROUP = 512               # rows per group iteration
NG = BN // GROUP          # 32

F32 = mybir.dt.float32
BF16 = mybir.dt.bfloat16
PI = math.pi


def build_kernel():
    nc = bacc.Bacc("TRN2", target_bir_lowering=False)
    x_d = nc.declare_dram_parameter("x", [BN, M2], F32, isOutput=False)
    pv_d = nc.declare_dram_parameter("pv", [64, 1], F32, isOutput=False)
    out_d = nc.declare_dram_parameter("out", [BN, P2], F32, isOutput=True)

    with tile.TileContext(nc) as tc:
        with (
            tc.tile_pool(name="const", bufs=1) as cpool,
            tc.tile_pool(name="xf", bufs=3) as xf_pool,
            tc.tile_pool(name="xbf", bufs=3) as xbf_pool,
            tc.tile_pool(name="xt", bufs=2) as xt_pool,
            tc.tile_pool(name="ytb", bufs=2) as ytb_pool,
            tc.tile_pool(name="ysb", bufs=3) as y_pool,
            tc.tile_pool(name="ps_xt", bufs=4, space="PSUM") as ps_xt,
            tc.tile_pool(name="ps_y", bufs=2, space="PSUM") as ps_y,
            tc.tile_pool(name="ps_yt", bufs=2, space="PSUM") as ps_yt,
        ):
            # ---------------- A generation ----------------
            # pv2[p] = p_values[p % 64], scaled by -2*pi/256
            pv2 = cpool.tile([128, 1], F32)
            nc.sync.dma_start(pv2[0:64, :], pv_d[:])
            nc.sync.dma_start(pv2[64:128, :], pv_d[:])
            # Work in "turns": u = -p_values[p]*m/256 + rowbias, then
            # range-reduce with the magic-number round trick and evaluate
            # Sin(2*pi*w) with w in [-0.5, 0.5].
            pvs = cpool.tile([128, 1], F32)
            nc.vector.tensor_scalar_mul(pvs[:], pv2[:], -1.0 / 256.0)

            m_i = cpool.tile([128, 256], mybir.dt.int32)
            nc.gpsimd.iota(m_i[:], pattern=[[1, 256]], base=0, channel_multiplier=0)
            m_f = cpool.tile([128, 256], F32)
            nc.vector.tensor_copy(m_f[:], m_i[:])

            # Row-dependent phase offsets in turns:
            #  left  half of A: rows<64 -> cos (0.25), rows>=64 -> sin (0)
            #  right half of A: rows<64 -> -sin (0.5), rows>=64 -> cos (0.25)
            bl = cpool.tile([128, 1], F32)
            nc.vector.memset(bl[0:64, :], 0.25)
            nc.vector.memset(bl[64:128, :], 0.0)
            br = cpool.tile([128, 1], F32)
            nc.vector.memset(br[0:64, :], 0.5)
            nc.vector.memset(br[64:128, :], 0.25)
            zero = cpool.tile([128, 1], F32)
            nc.vector.memset(zero[:], 0.0)

            MAGIC = 12582912.0  # 1.5 * 2**23: (u + MAGIC) - MAGIC == round(u)
            A_bf = cpool.tile([128, 512], BF16)
            for half, bias_ap in ((0, bl), (1, br)):
                u = cpool.tile([128, 256], F32, name=f"u{half}")
                nc.vector.tensor_scalar(
                    u[:], m_f[:], pvs[:], bias_ap[:],
                    mybir.AluOpType.mult, mybir.AluOpType.add,
                )
                t = cpool.tile([128, 256], F32, name=f"t{half}")
                nc.vector.tensor_scalar(
                    t[:], u[:], MAGIC, None, mybir.AluOpType.add
                )
                r = cpool.tile([128, 256], F32, name=f"r{half}")
                nc.vector.tensor_scalar(
                    r[:], t[:], MAGIC, None, mybir.AluOpType.subtract
                )
                w = cpool.tile([128, 256], F32, name=f"w{half}")
                nc.vector.tensor_tensor(
                    w[:], u[:], r[:], mybir.AluOpType.subtract
                )
                nc.scalar.activation(
                    A_bf[:, 256 * half : 256 * (half + 1)],
                    w[:],
                    mybir.ActivationFunctionType.Sin,
                    bias=zero[:],
                    scale=2.0 * PI,
                )

            ident = cpool.tile([128, 128], BF16)
            masks.make_identity(nc, ident[:])

            # AT[:, c, :] = transpose of A_bf[:, 128c:128c+128]  (lhsT chunks)
            AT = cpool.tile([128, 4, 128], BF16)
            for c in range(4):
                ps = ps_xt.tile([128, 4, 128], BF16, tag="xt_ps")
                nc.tensor.transpose(
                    ps[:, 0, :], A_bf[:, 128 * c : 128 * (c + 1)], ident[:]
                )
                nc.vector.tensor_copy(AT[:, c, :], ps[:, 0, :])

            # ---------------- main loop ----------------
            for g in range(NG):
                rows = slice(g * GROUP, (g + 1) * GROUP)
                xf = xf_pool.tile([128, 4, 512], F32)
                nc.sync.dma_start(
                    xf[:], x_d[rows, :].rearrange("(t p) m -> p t m", p=128)
                )
                xb = xbf_pool.tile([128, 4, 512], BF16)
                nc.scalar.copy(xb[:], xf[:])

                xt_ps = [
                    ps_xt.tile([128, 4, 128], BF16, tag="xt_ps", name=f"xt_ps{c}")
                    for c in range(4)
                ]
                for t in range(4):
                    for c in range(4):
                        nc.tensor.transpose(
                            xt_ps[c][:, t, :],
                            xb[:, t, 128 * c : 128 * (c + 1)],
                            ident[:],
                        )
                xts = xt_pool.tile([128, 4, 4, 128], BF16)  # [m_local, c, t, bn]
                for c in range(4):
                    nc.vector.tensor_copy(xts[:, c, :, :], xt_ps[c][:])

                yt_ps = ps_y.tile([128, 4, 128], F32)  # [p, t, bn]
                for c in range(4):
                    nc.tensor.matmul(
                        yt_ps[:],
                        AT[:, c, :],
                        xts[:, c, :, :],
                        start=(c == 0),
                        stop=(c == 3),
                    )

                ytb = ytb_pool.tile([128, 4, 128], BF16)
                nc.vector.tensor_copy(ytb[:], yt_ps[:])

                y_ps = ps_yt.tile([128, 4, 128], BF16)  # [bn, t, p]
                for t in range(4):
                    nc.tensor.transpose(y_ps[:, t, :], ytb[:, t, :], ident[:])

                ysb = y_pool.tile([128, 4, 128], F32)
                nc.scalar.copy(ysb[:], y_ps[:])
                nc.sync.dma_start(
                    out_d[rows, :].rearrange("(t p) m -> p t m", p=128), ysb[:]
                )

    nc.compile()
    return nc


_CACHE: dict = {}


def _get_nc():
    if "nc" not in _CACHE:
        _CACHE["nc"] = build_kernel()
    return _CACHE["nc"]


def _run(x, p_values, trace=False, **kw):
    nc = _get_nc()
    x = np.ascontiguousarray(x, dtype=np.float32)
    pv = np.ascontiguousarray(p_values, dtype=np.float32)
    in_maps = [
        {"x": x[c * SHARD : (c + 1) * SHARD].reshape(BN, M2), "pv": pv}
        for c in range(N_CORES)
    ]
    res = run_bass_kernel_spmd(
        nc, in_maps, core_ids=list(range(N_CORES)), trace=trace, **kw
    )
    outs = [r["out"].reshape(SHARD, NBLK * P2) for r in res.results]
    return np.concatenate(outs, axis=0), res


def kernel(x, p_values):
    out, _ = _run(x, p_values)
    return out


# revision 15
# speedup vs baseline: 1.1693x; 1.1693x over previous
"""Trainium2 Bass kernel for nn_AnalogLayer.

Math (see reference):
    A[p, m] built from cos/sin of (-2*pi/256 * p_values[p%64] * (m%256)),
    y[bn, :] = A @ x[bn, :]  for each of the batch*32 rows of length 512.

Strategy: pure data-parallel over batch across 8 NeuronCores. Per core the
shard is viewed as [16384, 512] rows; output is produced transposed
([128, 16384], column-major over rows) and fixed up on the host during the
gather/unshard step.

Per-core pipeline (per super-group of 2048 rows):
  - DMA x [128, 16, 512] f32 (partition = row%128, t = row/128, m)
  - cast to bf16 on ScalarE
  - VectorE StreamTranspose (32x32 blocks in place): partition 32a+r then
    holds m=32j+r for row-block a at free position (t, 32j+c), c=row%32
  - 16 j-chunks x 4 row-strips of K=32 matmuls (tile_position=(32a,0)),
    quad-concurrent in the PE array, lhsT = replicated A^T chunks,
    accumulating y^T[p, t, c] into one PSUM bank per strip
  - ScalarE copies PSUM -> SBUF f32, DMA out columns of [128, 16384]
"""

import math

import numpy as np

import concourse.bacc as bacc
import concourse.bass as bass
import concourse.masks as masks
import concourse.mybir as mybir
import concourse.tile as tile
from concourse.bass_utils import run_bass_kernel_spmd

N_CORES = 8
BATCH = 4096
SHARD = BATCH // N_CORES  # 512 batch rows per core
NBLK = 32                 # blocks per batch row
M2 = 512                  # contraction dim (2*M)
P2 = 128                  # output dim (2*P)
BN = SHARD * NBLK         # 16384 logical rows per core
SUPER = 2048              # rows per super-group (16 tiles of 128)
NT = SUPER // 128         # 16
NSG = BN // SUPER         # 8

F32 = mybir.dt.float32
BF16 = mybir.dt.bfloat16
PI = math.pi


def build_kernel():
    nc = bacc.Bacc("TRN2", target_bir_lowering=False)
    x_d = nc.declare_dram_parameter("x", [BN, M2], F32, isOutput=False)
    pv_d = nc.declare_dram_parameter("pv", [64, 1], F32, isOutput=False)
    out_d = nc.declare_dram_parameter("out", [P2, BN], F32, isOutput=True)

    with tile.TileContext(nc) as tc:
        with (
            tc.tile_pool(name="const", bufs=1) as cpool,
            tc.tile_pool(name="xf", bufs=2) as xf_pool,
            tc.tile_pool(name="xbf", bufs=2) as xbf_pool,
            tc.tile_pool(name="vt", bufs=2) as vt_pool,
            tc.tile_pool(name="ysb", bufs=2) as y_pool,
        ):
            # ---------------- A generation ----------------
            # Work in "turns": u = -p_values[p]*m/256 + rowbias, then
            # range-reduce with the magic-number round trick and evaluate
            # Sin(2*pi*w) with w in [-0.5, 0.5].
            pv2 = cpool.tile([128, 1], F32)
            nc.sync.dma_start(pv2[0:64, :], pv_d[:])
            nc.sync.dma_start(pv2[64:128, :], pv_d[:])
            pvs = cpool.tile([128, 1], F32)
            nc.vector.tensor_scalar_mul(pvs[:], pv2[:], -1.0 / 256.0)

            m_i = cpool.tile([128, 256], mybir.dt.int32)
            nc.gpsimd.iota(m_i[:], pattern=[[1, 256]], base=0, channel_multiplier=0)
            m_f = cpool.tile([128, 256], F32)
            nc.vector.tensor_copy(m_f[:], m_i[:])

            # Row-dependent phase offsets in turns:
            #  left  half of A: rows<64 -> cos (0.25), rows>=64 -> sin (0)
            #  right half of A: rows<64 -> -sin (0.5), rows>=64 -> cos (0.25)
            bl = cpool.tile([128, 1], F32)
            nc.vector.memset(bl[0:64, :], 0.25)
            nc.vector.memset(bl[64:128, :], 0.0)
            br = cpool.tile([128, 1], F32)
            nc.vector.memset(br[0:64, :], 0.5)
            nc.vector.memset(br[64:128, :], 0.25)
            zero = cpool.tile([128, 1], F32)
            nc.vector.memset(zero[:], 0.0)

            MAGIC = 12582912.0  # 1.5 * 2**23: (u + MAGIC) - MAGIC == round(u)
            A_bf = cpool.tile([128, 512], BF16)
            for half, bias_ap in ((0, bl), (1, br)):
                u = cpool.tile([128, 256], F32, name=f"u{half}")
                nc.vector.tensor_scalar(
                    u[:], m_f[:], pvs[:], bias_ap[:],
                    mybir.AluOpType.mult, mybir.AluOpType.add,
                )
                t = cpool.tile([128, 256], F32, name=f"t{half}")
                nc.vector.tensor_scalar(
                    t[:], u[:], MAGIC, None, mybir.AluOpType.add
                )
                r = cpool.tile([128, 256], F32, name=f"r{half}")
                nc.vector.tensor_scalar(
                    r[:], t[:], MAGIC, None, mybir.AluOpType.subtract
                )
                w = cpool.tile([128, 256], F32, name=f"w{half}")
                nc.vector.tensor_tensor(
                    w[:], u[:], r[:], mybir.AluOpType.subtract
                )
                nc.scalar.activation(
                    A_bf[:, 256 * half : 256 * (half + 1)],
                    w[:],
                    mybir.ActivationFunctionType.Sin,
                    bias=zero[:],
                    scale=2.0 * PI,
                )

            ident = cpool.tile([128, 128], BF16)
            masks.make_identity(nc, ident[:])

            # AT_sb[mu, c, p] = A[p, 128c + mu]  (PE transpose of A chunks)
            AT_sb = cpool.tile([128, 4, 128], BF16)
            with tc.tile_pool(name="ps_at", bufs=2, space="PSUM") as ps_at:
                for c in range(4):
                    at_ps = ps_at.tile(
                        [128, 128], BF16, tag="at_ps", name=f"at_ps{c}"
                    )
                    nc.tensor.transpose(
                        at_ps[:], A_bf[:, 128 * c : 128 * (c + 1)], ident[:]
                    )
                    nc.vector.tensor_copy(AT_sb[:, c, :], at_ps[:])

            # Replicate A^T to all 4 partition strips via a DRAM bounce:
            # at_dram[r, j, p] = A[p, 32j + r]  (j = 4c + jj)
            with tc.tile_pool(name="dram", bufs=1, space="DRAM") as dpool:
                # layout [r, c, jj, p]: j = 4c + jj is contiguous as (c, jj)
                at_dram = dpool.tile([32, 4, 4, 128], BF16)
                for jj in range(4):
                    nc.sync.dma_start(
                        at_dram[:, :, jj, :],
                        AT_sb[32 * jj : 32 * (jj + 1), :, :],
                    )
                # ATrep[32a + r, j, p] = A[p, 32j + r]
                ATrep = cpool.tile([128, 16, 128], BF16)
                for a in range(4):
                    nc.sync.dma_start(
                        ATrep[32 * a : 32 * (a + 1), :, :],
                        at_dram[:].rearrange("r c jj p -> r (c jj) p"),
                    )

            # ---------------- main loop ----------------
            ps_cm = tc.tile_pool(name="ps", bufs=8, space="PSUM")
            ps = ps_cm.__enter__()
            for sg in range(NSG):
                rows = slice(sg * SUPER, (sg + 1) * SUPER)
                xf = xf_pool.tile([128, NT, 512], F32)
                nc.sync.dma_start(
                    xf[:], x_d[rows, :].rearrange("(t p) m -> p t m", p=128)
                )
                xb = xbf_pool.tile([128, NT, 512], BF16)
                nc.scalar.copy(xb[:], xf[:])
                v = vt_pool.tile([128, NT, 512], BF16)
                nc.vector.transpose(v[:], xb[:])

                yts = [
                    ps.tile([128, NT, 32], F32, tag="yt", name=f"yt{a}")
                    for a in range(4)
                ]
                for j in range(16):
                    for a in range(4):
                        nc.tensor.matmul(
                            yts[a][:],
                            ATrep[32 * a : 32 * (a + 1), j, :],
                            v[32 * a : 32 * (a + 1), :, 32 * j : 32 * (j + 1)],
                            start=(j == 0),
                            stop=(j == 15),
                            tile_position=(32 * a, 0),
                        )

                ysb = y_pool.tile([128, NT, 4, 32], F32)
                for a in range(4):
                    nc.scalar.copy(ysb[:, :, a, :], yts[a][:])
                nc.sync.dma_start(
                    out_d[:, rows], ysb[:].rearrange("p t a c -> p (t a c)")
                )
            ps_cm.__exit__(None, None, None)

    nc.compile()
    return nc


_CACHE: dict = {}


def _get_nc():
    if "nc" not in _CACHE:
        _CACHE["nc"] = build_kernel()
    return _CACHE["nc"]


def _run(x, p_values, trace=False, **kw):
    nc = _get_nc()
    x = np.ascontiguousarray(x, dtype=np.float32)
    pv = np.ascontiguousarray(p_values, dtype=np.float32)
    in_maps = [
        {"x": x[c * SHARD : (c + 1) * SHARD].reshape(BN, M2), "pv": pv}
        for c in range(N_CORES)
    ]
    res = run_bass_kernel_spmd(
        nc, in_maps, core_ids=list(range(N_CORES)), trace=trace, **kw
    )
    out = np.empty((BATCH, NBLK * P2), dtype=np.float32)
    for c in range(N_CORES):
        # res is [128, 16384] = y_core^T; un-transpose during the gather
        out[c * SHARD : (c + 1) * SHARD] = (
            res.results[c]["out"].T.reshape(SHARD, NBLK * P2)
        )
    return out, res


def kernel(x, p_values):
    out, _ = _run(x, p_values)
    return out


# revision 16
# speedup vs baseline: 1.2022x; 1.0282x over previous
"""Trainium2 Bass kernel for nn_AnalogLayer.

Math (see reference):
    A[p, m] built from cos/sin of (-2*pi/256 * p_values[p%64] * (m%256)),
    y[bn, :] = A @ x[bn, :]  for each of the batch*32 rows of length 512.

Strategy: pure data-parallel over batch across 8 NeuronCores. Per core the
shard is viewed as [16384, 512] rows; output is produced transposed
([128, 16384] bf16) and fixed up on the host during the gather/unshard step.

Per-core pipeline (per super-group of 2048 rows):
  - gpsimd cast-DMA x -> SBUF bf16 [128, 16, 512] (partition = row%128)
  - VectorE StreamTranspose (32x32 blocks in place, 4 chunks): partition
    32a+r then holds m=32j+r for row-block a at free position (t, 32j+c)
  - 16 j-chunks x 4 row-strips of K=32 matmuls (tile_position=(32a,0)),
    quad-concurrent in the PE array, lhsT = replicated A^T chunks,
    accumulating y^T[p, t, c] into one PSUM bank per strip
  - ScalarE copies PSUM -> SBUF bf16, DMA out columns of [128, 16384]
"""

import math

import numpy as np

import concourse.bacc as bacc
import concourse.bass as bass
import concourse.masks as masks
import concourse.mybir as mybir
import concourse.tile as tile
from concourse.bass_utils import run_bass_kernel_spmd

N_CORES = 8
BATCH = 4096
SHARD = BATCH // N_CORES  # 512 batch rows per core
NBLK = 32                 # blocks per batch row
M2 = 512                  # contraction dim (2*M)
P2 = 128                  # output dim (2*P)
BN = SHARD * NBLK         # 16384 logical rows per core
SUPER = 2048              # rows per super-group (16 tiles of 128)
NT = SUPER // 128         # 16
NSG = BN // SUPER         # 8

F32 = mybir.dt.float32
BF16 = mybir.dt.bfloat16
PI = math.pi


def build_kernel():
    nc = bacc.Bacc("TRN2", target_bir_lowering=False)
    x_d = nc.declare_dram_parameter("x", [BN, M2], F32, isOutput=False)
    pv_d = nc.declare_dram_parameter("pv", [64, 1], F32, isOutput=False)
    out_d = nc.declare_dram_parameter("out", [P2, BN], BF16, isOutput=True)

    with tile.TileContext(nc) as tc:
        with (
            tc.tile_pool(name="const", bufs=1) as cpool,
            tc.tile_pool(name="xbf", bufs=3) as xbf_pool,
            tc.tile_pool(name="vt", bufs=3) as vt_pool,
            tc.tile_pool(name="ysb", bufs=3) as y_pool,
        ):
            # ---------------- A generation ----------------
            # Work in "turns": u = -p_values[p]*m/256 + rowbias, then
            # range-reduce with the magic-number round trick and evaluate
            # Sin(2*pi*w) with w in [-0.5, 0.5].
            pv2 = cpool.tile([128, 1], F32)
            nc.sync.dma_start(pv2[0:64, :], pv_d[:])
            nc.sync.dma_start(pv2[64:128, :], pv_d[:])
            pvs = cpool.tile([128, 1], F32)
            nc.vector.tensor_scalar_mul(pvs[:], pv2[:], -1.0 / 256.0)

            m_i = cpool.tile([128, 256], mybir.dt.int32)
            nc.gpsimd.iota(m_i[:], pattern=[[1, 256]], base=0, channel_multiplier=0)
            m_f = cpool.tile([128, 256], F32)
            nc.vector.tensor_copy(m_f[:], m_i[:])

            # Row-dependent phase offsets in turns:
            #  left  half of A: rows<64 -> cos (0.25), rows>=64 -> sin (0)
            #  right half of A: rows<64 -> -sin (0.5), rows>=64 -> cos (0.25)
            bl = cpool.tile([128, 1], F32)
            nc.vector.memset(bl[0:64, :], 0.25)
            nc.vector.memset(bl[64:128, :], 0.0)
            br = cpool.tile([128, 1], F32)
            nc.vector.memset(br[0:64, :], 0.5)
            nc.vector.memset(br[64:128, :], 0.25)
            zero = cpool.tile([128, 1], F32)
            nc.vector.memset(zero[:], 0.0)

            MAGIC = 12582912.0  # 1.5 * 2**23: (u + MAGIC) - MAGIC == round(u)
            A_bf = cpool.tile([128, 512], BF16)
            for half, bias_ap in ((0, bl), (1, br)):
                u = cpool.tile([128, 256], F32, name=f"u{half}")
                nc.vector.tensor_scalar(
                    u[:], m_f[:], pvs[:], bias_ap[:],
                    mybir.AluOpType.mult, mybir.AluOpType.add,
                )
                t = cpool.tile([128, 256], F32, name=f"t{half}")
                nc.vector.tensor_scalar(
                    t[:], u[:], MAGIC, None, mybir.AluOpType.add
                )
                r = cpool.tile([128, 256], F32, name=f"r{half}")
                nc.vector.tensor_scalar(
                    r[:], t[:], MAGIC, None, mybir.AluOpType.subtract
                )
                w = cpool.tile([128, 256], F32, name=f"w{half}")
                nc.vector.tensor_tensor(
                    w[:], u[:], r[:], mybir.AluOpType.subtract
                )
                nc.scalar.activation(
                    A_bf[:, 256 * half : 256 * (half + 1)],
                    w[:],
                    mybir.ActivationFunctionType.Sin,
                    bias=zero[:],
                    scale=2.0 * PI,
                )

            ident = cpool.tile([128, 128], BF16)
            masks.make_identity(nc, ident[:])

            # AT_sb[mu, c, p] = A[p, 128c + mu]  (PE transpose of A chunks)
            AT_sb = cpool.tile([128, 4, 128], BF16)
            with tc.tile_pool(name="ps_at", bufs=2, space="PSUM") as ps_at:
                for c in range(4):
                    at_ps = ps_at.tile(
                        [128, 128], BF16, tag="at_ps", name=f"at_ps{c}"
                    )
                    nc.tensor.transpose(
                        at_ps[:], A_bf[:, 128 * c : 128 * (c + 1)], ident[:]
                    )
                    nc.vector.tensor_copy(AT_sb[:, c, :], at_ps[:])

            # Replicate A^T to all 4 partition strips via a DRAM bounce:
            # at_dram[r, c, jj, p]: j = 4c + jj, value A[p, 32j + r]
            with tc.tile_pool(name="dram", bufs=1, space="DRAM") as dpool:
                at_dram = dpool.tile([32, 4, 4, 128], BF16)
                for jj in range(4):
                    nc.sync.dma_start(
                        at_dram[:, :, jj, :],
                        AT_sb[32 * jj : 32 * (jj + 1), :, :],
                    )
                # ATrep[32a + r, j, p] = A[p, 32j + r]
                ATrep = cpool.tile([128, 16, 128], BF16)
                for a in range(4):
                    nc.sync.dma_start(
                        ATrep[32 * a : 32 * (a + 1), :, :],
                        at_dram[:].rearrange("r c jj p -> r (c jj) p"),
                    )

            # ---------------- main loop ----------------
            ps_cm = tc.tile_pool(name="ps", bufs=8, space="PSUM")
            ps = ps_cm.__enter__()
            for sg in range(NSG):
                rows = slice(sg * SUPER, (sg + 1) * SUPER)
                xb = xbf_pool.tile([128, NT, 512], BF16)
                nc.gpsimd.dma_start(
                    xb[:], x_d[rows, :].rearrange("(t p) m -> p t m", p=128)
                )
                v = vt_pool.tile([128, NT, 512], BF16)
                for q in range(4):
                    nc.vector.transpose(
                        v[:, 4 * q : 4 * (q + 1), :], xb[:, 4 * q : 4 * (q + 1), :]
                    )

                yts = [
                    ps.tile([128, NT, 32], F32, tag="yt", name=f"yt{a}")
                    for a in range(4)
                ]
                for j in range(16):
                    for a in range(4):
                        nc.tensor.matmul(
                            yts[a][:],
                            ATrep[32 * a : 32 * (a + 1), j, :],
                            v[32 * a : 32 * (a + 1), :, 32 * j : 32 * (j + 1)],
                            start=(j == 0),
                            stop=(j == 15),
                            tile_position=(32 * a, 0),
                        )

                ysb = y_pool.tile([128, NT, 4, 32], BF16)
                for a in range(4):
                    nc.scalar.copy(ysb[:, :, a, :], yts[a][:])
                nc.sync.dma_start(
                    out_d[:, rows], ysb[:].rearrange("p t a c -> p (t a c)")
                )
            ps_cm.__exit__(None, None, None)

    nc.compile()
    return nc


_CACHE: dict = {}


def _get_nc():
    if "nc" not in _CACHE:
        _CACHE["nc"] = build_kernel()
    return _CACHE["nc"]


def _run(x, p_values, trace=False, **kw):
    nc = _get_nc()
    x = np.ascontiguousarray(x, dtype=np.float32)
    pv = np.ascontiguousarray(p_values, dtype=np.float32)
    in_maps = [
        {"x": x[c * SHARD : (c + 1) * SHARD].reshape(BN, M2), "pv": pv}
        for c in range(N_CORES)
    ]
    res = run_bass_kernel_spmd(
        nc, in_maps, core_ids=list(range(N_CORES)), trace=trace, **kw
    )
    out = np.empty((BATCH, NBLK * P2), dtype=np.float32)
    for c in range(N_CORES):
        # res is [128, 16384] bf16 = y_core^T; un-transpose during the gather
        out[c * SHARD : (c + 1) * SHARD] = (
            res.results[c]["out"].astype(np.float32).T.reshape(SHARD, NBLK * P2)
        )
    return out, res


def kernel(x, p_values):
    out, _ = _run(x, p_values)
    return out
